# revision 31
# baseline (speedup 1.0000x reference)
"""FDGNN (gnn_message_passing) Trainium2 kernel, 8-core SPMD.

Strategy (v2 — deep pipelining):
- Only 3 of the reference's 6 convs feed the output:
    s1 = conv_i2s(xi0); i2 = conv_s2i(s1); s3 = conv_i2s(i2); out = tanh(s3@wo+bo)
- mlp_m commutes with the per-edge gather: mlp_m(x[src]) == mlp_m(x)[src], so
  the message MLP runs per *node* (12.5K rows/core), not per *edge*.
- Destination nodes are sharded across the 8 cores.
- The message table is split into 4 source-quarters (window-aligned, int16-
  indexable); each quarter is AllGathered separately so gathers for quarter q
  start while later quarters are still in flight.
- Segment-sum: a window-group of 24-26 dst windows stays resident in PSUM
  while all 4 quarter streams accumulate into it:
      psum_w[feat(dup), dst128] += gathered_tile.T @ S_tile
  S_tile is a 0/1 dst-selection matrix built on DVE via iota compare.
- After each window-group: mlp_u for those columns, the NEXT conv's mlp_m for
  the same columns, and the next conv's quarter-AllGather — so collectives and
  MLPs hide behind the next group's gather/matmul work.
- MLPs run in bf16 (fp32 PSUM accumulate).
"""

import numpy as np
import os as _os

NCORES = 8
NNODE = 100000  # both NS and NI
PERCORE = NNODE // NCORES  # 12500
NW = 98  # windows per core (98*128 = 12544)
PADPER = NW * 128  # 12544 padded rows per core
NQ = 4  # source quarters (table chunks)
QSTART = [0, 3072, 6144, 9216, 12544]
QSIZE = [3072, 3072, 3072, 3328]
CHUNK8 = [8 * s for s in QSIZE]  # per-quarter table rows (< 32768, int16-safe)
# dst window-groups kept resident in PSUM (PSUM is bank-granular: 1 bank per
# [128,128] fp32 tile, 8 banks total -> 4 windows/group = one 512-col tile)
GROUPS = [(4 * g, 4 * g + 4) for g in range(24)] + [(96, 98)]
CT_OF_GROUP = [(g, g + 1) for g in range(25)]
AG_AFTER_GROUP = {5: 0, 11: 1, 17: 2, 24: 3}  # group -> quarter to AllGather
CT_RANGE = [(0, 6), (6, 12), (12, 18), (18, 25)]  # 512-col tiles per quarter
D = 64
HM = 32
HU = 16

GT = int(_os.environ.get("KGT", "20"))  # tiles per dma_gather call
SB = 8  # tiles per S-build batch

TRACE = False  # set by test harness to capture an NTFF profile
LAST_RESULT = None  # BassKernelResults of the most recent run

# PADPER = 12544 = 24*512 + 256
COL_TILES = [(i * 512, 512) for i in range(PADPER // 512)]
if PADPER % 512:
    COL_TILES.append((PADPER - PADPER % 512, PADPER % 512))


# ---------------------------------------------------------------- host prep

def _prep_relation(src, dst):
    """Route edges (dst-sharded) into per-core, per-quarter gather streams."""
    E = src.shape[0]
    src = src.astype(np.int64)
    dst = dst.astype(np.int64)
    qstart = np.asarray(QSTART[:NQ])
    qsize = np.asarray(QSIZE)

    p = dst // PERCORE
    dl = dst - p * PERCORE
    w = dl >> 7
    drel = dl & 127

    sl = src % PERCORE
    sc = src // PERCORE
    q = np.searchsorted(QSTART[1:], sl, side="right")
    lidx = sc * qsize[q] + (sl - qstart[q])

    key = (p * NQ + q) * NW + w
    counts = np.bincount(key, minlength=NCORES * NQ * NW).reshape(NCORES, NQ, NW)
    ntiles_qw = -(-counts.max(axis=0) // 128)  # [NQ, NW]
    ntiles_qw[0] = np.maximum(ntiles_qw[0], 1)  # every window has >=1 tile
    N_qw = ntiles_qw * 128
    base_w = np.zeros((NQ, NW + 1), np.int64)
    base_w[:, 1:] = np.cumsum(N_qw, axis=1)
    T_q = (base_w[:, -1] // 128).astype(np.int64)  # tiles per quarter stream

    # rank of each edge within its (p, q, w) cell
    order = np.argsort(key, kind="stable")
    kk = key[order]
    grp_first = np.r_[True, kk[1:] != kk[:-1]]
    first_pos = np.flatnonzero(grp_first)
    starts = np.repeat(first_pos, np.diff(np.r_[first_pos, E]))
    rank = np.arange(E) - starts
    inv = np.empty(E, np.int64)
    inv[order] = rank
    slot = base_w[q, w] + inv  # slot within (core, quarter) stream

    idx_streams = []  # [core][q] -> int16 [128, T_q*8] packed
    drel_streams = []  # [core][q] -> fp32 [128, T_q]
    for pp in range(NCORES):
        rows_i = []
        rows_d = []
        pm = p == pp
        for qq in range(NQ):
            n = int(T_q[qq]) * 128
            ar = np.arange(n)
            idx_flat = ((ar >> 7) * 197) % CHUNK8[qq]  # one dummy row per tile
            drel_flat = np.full(n, -1.0, np.float32)
            m = pm & (q == qq)
            idx_flat[slot[m]] = lidx[m]
            drel_flat[slot[m]] = drel[m]
            assert idx_flat.max() < CHUNK8[qq] and idx_flat.min() >= 0
            idx16 = idx_flat.astype(np.int16)
            packed = np.tile(idx16.reshape(n // 16, 16).T, (8, 1))  # [128, n/16]
            rows_i.append(np.ascontiguousarray(packed))
            rows_d.append(
                np.ascontiguousarray(drel_flat.reshape(-1, 128).T.astype(np.float32))
            )
        idx_streams.append(rows_i)
        drel_streams.append(rows_d)

    return {
        "ntiles_qw": ntiles_qw,  # [NQ, NW]
        "T_q": T_q,  # [NQ]
        "idx": idx_streams,
        "drel": drel_streams,
    }


# ---------------------------------------------------------------- program

def _build_program(meta_a, meta_b):
    """meta_a: i2s relation (convs 1 and 3), meta_b: s2i relation (conv 2)."""
    import concourse.mybir as mybir
    import concourse.tile as tile
    from concourse import bacc
    from concourse.bass import ts
    import ml_dtypes

    FP32 = mybir.dt.float32
    BF16 = mybir.dt.bfloat16
    I16 = mybir.dt.int16
    AF = mybir.ActivationFunctionType

    nc = bacc.Bacc(
        "TRN2",
        target_bir_lowering=False,
        debug=False,
        enable_asserts=False,
        num_devices=NCORES,
        num_swdge_queues=4,
        dynamic_dma_scratch_size=49152,
    )

    # ---- I/O (weights bf16, biases fp32)
    xi0T = nc.dram_tensor("xi0T", [D + 1, PADPER], BF16, kind="ExternalInput")
    wm1 = nc.dram_tensor("wm1", [D, HM], BF16, kind="ExternalInput")
    bm1 = nc.dram_tensor("bm1", [HM, 1], FP32, kind="ExternalInput")
    wm2 = nc.dram_tensor("wm2", [HM, D], BF16, kind="ExternalInput")
    bm2r = nc.dram_tensor("bm2r", [1, D], BF16, kind="ExternalInput")
    wu1 = nc.dram_tensor("wu1", [D, HU], BF16, kind="ExternalInput")
    bu1 = nc.dram_tensor("bu1", [HU, 1], FP32, kind="ExternalInput")
    wu2 = nc.dram_tensor("wu2", [HU, D], BF16, kind="ExternalInput")
    bu2 = nc.dram_tensor("bu2", [D, 1], FP32, kind="ExternalInput")
    wob = nc.dram_tensor("wob", [D + 1, D], BF16, kind="ExternalInput")

    idx_in = {}
    drel_in = {}
    for rel, meta in (("a", meta_a), ("b", meta_b)):
        for qq in range(NQ):
            tqn = int(meta["T_q"][qq])
            idx_in[rel, qq] = nc.dram_tensor(
                f"idx_{rel}{qq}", [128, tqn * 8], I16, kind="ExternalInput"
            )
            drel_in[rel, qq] = nc.dram_tensor(
                f"drel_{rel}{qq}", [128, tqn], FP32, kind="ExternalInput"
            )

    out = nc.dram_tensor("out", [PADPER, D], FP32, kind="ExternalOutput")

    # collective buffers; rows hold the 64 bf16 features twice (256B granule)
    y_bounce = nc.dram_tensor("y_bounce", [PADPER, 2 * D], BF16)
    tables = {
        rel: [
            nc.dram_tensor(f"table_{rel}{q}", [CHUNK8[q], 2 * D], BF16,
                           addr_space="Shared")
            for q in range(NQ)
        ]
        for rel in ("a", "b")
    }

    iota_np = np.tile(np.arange(128, dtype=np.float32), (128, 1))
    iota_dram = nc.inline_tensor(iota_np, name="iota")
    onescol_dram = nc.inline_tensor(
        np.ones((1, 512), ml_dtypes.bfloat16), name="onescol"
    )
    zeros_dram = nc.inline_tensor(
        np.zeros((44, 2 * D), ml_dtypes.bfloat16), name="zerorows"
    )

    with tile.TileContext(nc) as tc:
        with (
            tc.tile_pool(name="consts", bufs=1) as cs,
            tc.tile_pool(name="state", bufs=1) as st,
            tc.tile_pool(name="stage", bufs=3) as sg,
            tc.tile_pool(name="meta", bufs=5) as mp,
            tc.tile_pool(name="g0", bufs=3) as gp0,
            tc.tile_pool(name="g1", bufs=3) as gp1,
            tc.tile_pool(name="g2", bufs=3) as gp2,
            tc.tile_pool(name="g3", bufs=3) as gp3,
            tc.tile_pool(name="spool", bufs=2) as sp,
            tc.tile_pool(name="pw", bufs=1, space="PSUM") as pw,
            tc.tile_pool(name="pa", bufs=1, space="PSUM") as pa,
            tc.tile_pool(name="pb", bufs=2, space="PSUM") as pb,
        ):
            gpools = [gp0, gp1, gp2, gp3]
            # ---- constants
            iota_s = cs.tile([128, 128], FP32)
            nc.sync.dma_start(out=iota_s[:], in_=iota_dram[:, :])
            wm1_s = cs.tile([D, HM], BF16)
            nc.sync.dma_start(out=wm1_s[:], in_=wm1[:, :])
            bm1_s = cs.tile([HM, 1], FP32)
            nc.sync.dma_start(out=bm1_s[:], in_=bm1[:, :])
            wm2_s = cs.tile([HM, D], BF16)
            nc.sync.dma_start(out=wm2_s[:], in_=wm2[:, :])
            bm2r_s = cs.tile([1, D], BF16)
            nc.sync.dma_start(out=bm2r_s[:], in_=bm2r[:, :])
            onescol_s = cs.tile([1, 512], BF16)
            nc.sync.dma_start(out=onescol_s[:], in_=onescol_dram[:, :])
            wu1_s = cs.tile([D, HU], BF16)
            nc.sync.dma_start(out=wu1_s[:], in_=wu1[:, :])
            bu1_s = cs.tile([HU, 1], FP32)
            nc.sync.dma_start(out=bu1_s[:], in_=bu1[:, :])
            wu2_s = cs.tile([HU, D], BF16)
            nc.sync.dma_start(out=wu2_s[:], in_=wu2[:, :])
            bu2_s = cs.tile([D, 1], FP32)
            nc.sync.dma_start(out=bu2_s[:], in_=bu2[:, :])
            wob_s = cs.tile([D + 1, D], BF16)
            nc.sync.dma_start(out=wob_s[:], in_=wob[:, :])

            # ---- persistent state
            xT = st.tile([D + 1, PADPER], BF16)  # row D = ones
            nc.sync.dma_start(out=xT[:], in_=xi0T[:, :])
            aggT = st.tile([D, PADPER], BF16)

            # zero rows of the y bounce buffer (pad rows 12500..12543), once
            zrow = cs.tile([44, 2 * D], BF16)
            nc.sync.dma_start(out=zrow[:], in_=zeros_dram[:, :])
            nc.sync.dma_start(out=y_bounce[12500:PADPER, :], in_=zrow[:])

            h1_of = {}

            def mlp_m1_ct(ct):
                """First half of mlp_m for col tile ct: h1 = relu(wm1.T@x+b)."""
                c0, cn = COL_TILES[ct]
                ps = pa.tile([HM, 512], FP32, tag="pa")
                nc.tensor.matmul(
                    ps[:, :cn], wm1_s[:], xT[0:D, c0 : c0 + cn],
                    start=True, stop=True,
                )
                h1 = sg.tile([HM, 512], BF16, tag="h1")
                nc.scalar.activation(h1[:, :cn], ps[:, :cn], AF.Relu, bias=bm1_s[:])
                h1_of[ct] = h1

            def mlp_m2_ct(ct):
                """Second half: y_bounce windows of ct (row-major via flip)."""
                c0, cn = COL_TILES[ct]
                h1 = h1_of.pop(ct)
                for j0 in range(0, cn, 128):
                    j = (c0 + j0) // 128
                    ps2 = pb.tile([128, D], FP32, tag="pb")
                    # bias via rank-1 outer product: ones[128] x bm2[64]
                    nc.tensor.matmul(
                        ps2[:], onescol_s[:, 0:128], bm2r_s[:],
                        start=True, stop=False,
                    )
                    nc.tensor.matmul(
                        ps2[:], h1[:, j0 : j0 + 128], wm2_s[:],
                        start=False, stop=True,
                    )
                    ydup = sg.tile([128, 2 * D], BF16, tag="ydup")
                    nc.scalar.activation(ydup[:, 0:D], ps2[:], AF.Relu)
                    nc.scalar.activation(ydup[:, D : 2 * D], ps2[:], AF.Relu)
                    r0 = j * 128
                    nrows = 128 if j < NW - 1 else (12500 - r0)
                    nc.sync.dma_start(
                        out=y_bounce[r0 : r0 + nrows, :], in_=ydup[0:nrows, :]
                    )

            def fire_ag(rel, q):
                nc.gpsimd.collective_compute(
                    "AllGather",
                    mybir.AluOpType.bypass,
                    replica_groups=[list(range(NCORES))],
                    ins=[y_bounce[QSTART[q] : QSTART[q + 1], :].opt()],
                    outs=[tables[rel][q].ap().opt()],
                )

            hu_of = {}

            def mlp_u1_ct(ct):
                """hu = relu(wu1.T @ aggT[ct cols] + bu1)."""
                c0, cn = COL_TILES[ct]
                ps1 = pa.tile([HU, 512], FP32, tag="pa")
                nc.tensor.matmul(
                    ps1[:, :cn], wu1_s[:], aggT[:, c0 : c0 + cn],
                    start=True, stop=True,
                )
                hu = sg.tile([HU, 512], BF16, tag="hu")
                nc.scalar.activation(hu[:, :cn], ps1[:, :cn], AF.Relu, bias=bu1_s[:])
                hu_of[ct] = hu

            def mlp_u2_ct(ct):
                """xT[:, ct cols] = relu(wu2.T @ hu + bu2)."""
                c0, cn = COL_TILES[ct]
                hu = hu_of.pop(ct)
                ps2 = pa.tile([D, 512], FP32, tag="pu2")
                nc.tensor.matmul(
                    ps2[:, :cn], wu2_s[:], hu[:, :cn], start=True, stop=True
                )
                nc.scalar.activation(
                    xT[0:D, c0 : c0 + cn], ps2[:, :cn], AF.Relu, bias=bu2_s[:]
                )

            def h2o_ct(ct):
                c0, cn = COL_TILES[ct]
                for j0 in range(0, cn, 128):
                    j = (c0 + j0) // 128
                    ps = pb.tile([128, D], FP32, tag="pb")
                    nc.tensor.matmul(
                        ps[:], xT[:, ts(j, 128)], wob_s[:], start=True, stop=True
                    )
                    ostage = sg.tile([128, D], FP32, tag="ostage")
                    nc.scalar.activation(ostage[:], ps[:], AF.Tanh)
                    nc.sync.dma_start(out=out[ts(j, 128), :], in_=ostage[:])

            call_counter = [0]

            def conv(meta, rel, stages, ag_at):
                """Gather + segment-sum -> aggT, with software-pipelined tails.

                stages: list of (offset, fn); after group g emit fn(g - offset)
                        in the given order (put oldest/ready work first).
                ag_at: dict {pipe_step: thunk} firing next conv's AllGathers.
                """

                def pipe_step(g):
                    for off, fn in stages:
                        ct = g - off
                        if 0 <= ct < len(COL_TILES):
                            fn(ct)
                    if g in ag_at:
                        ag_at[g]()
                ntiles_qw = meta["ntiles_qw"]
                T_q = meta["T_q"]
                tbl = tables[rel]

                drel_s = []
                for qq in range(NQ):
                    tqn = int(T_q[qq])
                    dt_ = mp.tile([128, tqn], FP32, tag=f"drel{qq}")
                    nc.sync.dma_start(out=dt_[:], in_=drel_in[rel, qq][:, :])
                    drel_s.append(dt_)

                calls = []
                for qq in range(NQ):
                    tqn = int(T_q[qq])
                    calls.append(
                        [(t0, min(GT, tqn - t0)) for t0 in range(0, tqn, GT)]
                    )

                # per-quarter cumulative tile needs through each group
                need = np.zeros((NQ, len(GROUPS)), np.int64)
                for g, (w0, w1) in enumerate(GROUPS):
                    for qq in range(NQ):
                        need[qq, g] = ntiles_qw[qq, w0:w1].sum()
                need = np.cumsum(need, axis=1)

                ghandles = [dict() for _ in range(NQ)]
                shandles = [dict() for _ in range(NQ)]
                ixhandles = [dict() for _ in range(NQ)]
                next_call = [0] * NQ
                next_ix = [0] * NQ
                next_sb = [0] * NQ

                def issue_idx(qq):
                    k = next_ix[qq]
                    t0, nt = calls[qq][k]
                    ix = mp.tile([128, nt * 8], I16, tag=f"idx{qq}")
                    nc.sync.dma_start(
                        out=ix[:],
                        in_=idx_in[rel, qq][:, t0 * 8 : (t0 + nt) * 8],
                    )
                    ixhandles[qq][k] = ix
                    ixhandles[qq].pop(k - 5, None)
                    next_ix[qq] = k + 1

                def issue_gather(qq):
                    k = next_call[qq]
                    t0, nt = calls[qq][k]
                    ix = ixhandles[qq][k]
                    gb = gpools[qq].tile([128, nt, 2 * D], BF16, tag=f"gb{qq}")
                    nc.gpsimd.dma_gather(
                        gb[:],
                        tbl[qq][:, :],
                        ix[:],
                        nt * 128,
                        nt * 128,
                        2 * D,
                        elem_step=2 * D,
                        queue_num=qq,
                        single_packet=False,
                    )
                    call_counter[0] += 1
                    ghandles[qq][k] = gb
                    ghandles[qq].pop(k - 4, None)
                    next_call[qq] = k + 1

                def issue_s(qq):
                    k = next_sb[qq]
                    t0 = k * SB
                    nb = min(SB, int(T_q[qq]) - t0)
                    stile = sp.tile([128, SB, 128], BF16, tag=f"sb{qq}")
                    nc.vector.tensor_tensor(
                        out=stile[:, 0:nb, :],
                        in0=drel_s[qq][:, t0 : t0 + nb].to_broadcast(
                            [128, nb, 128]
                        ),
                        in1=iota_s[:]
                        .rearrange("p (o w) -> p o w", o=1)
                        .to_broadcast([128, nb, 128]),
                        op=mybir.AluOpType.is_equal,
                    )
                    shandles[qq][k] = stile
                    shandles[qq].pop(k - 5, None)
                    next_sb[qq] = k + 1

                def prefetch_gathers(g):
                    tgt_g = min(g + 1, len(GROUPS) - 1)
                    tgt_ix = min(g + 2, len(GROUPS) - 1)
                    for qq in range(NQ):
                        while (
                            next_ix[qq] < len(calls[qq])
                            and next_ix[qq] * GT < need[qq, tgt_ix] + GT
                        ):
                            issue_idx(qq)
                        while (
                            next_call[qq] < len(calls[qq])
                            and next_call[qq] * GT < need[qq, tgt_g]
                        ):
                            issue_gather(qq)

                def prefetch_s(g):
                    tgt_g = min(g + 1, len(GROUPS) - 1)
                    nbat = [
                        -(-int(T_q[qq]) // SB) for qq in range(NQ)
                    ]
                    for qq in range(NQ):
                        while (
                            next_sb[qq] < nbat[qq]
                            and next_sb[qq] * SB < need[qq, tgt_g]
                        ):
                            issue_s(qq)

                prefetch_s(-1)  # group-0 S batches
                tile_cursor = [0] * NQ
                for g, (w0, w1) in enumerate(GROUPS):
                    prefetch_gathers(g)
                    psw = {
                        w: pw.tile(
                            [128, 128], FP32, tag=f"ps{w - w0}",
                            name=f"psw{w - w0}",
                        )
                        for w in range(w0, w1)
                    }
                    total = {
                        w: int(ntiles_qw[:, w].sum()) for w in range(w0, w1)
                    }
                    mm = {w: 0 for w in range(w0, w1)}
                    for qq in range(NQ):
                        t = tile_cursor[qq]
                        for w in range(w0, w1):
                            for _ in range(int(ntiles_qw[qq, w])):
                                gk = t // GT
                                gb = ghandles[qq][gk]
                                gslot = t - calls[qq][gk][0]
                                sk = t // SB
                                stile = shandles[qq][sk]
                                sslot = t - sk * SB
                                nc.tensor.matmul(
                                    psw[w][:],
                                    gb[:, gslot, :],
                                    stile[:, sslot, :],
                                    start=(mm[w] == 0),
                                    stop=(mm[w] == total[w] - 1),
                                )
                                t += 1
                                mm[w] += 1
                        tile_cursor[qq] = t
                    for w in range(w0, w1):
                        nc.vector.tensor_copy(
                            out=aggT[:, ts(w, 128)], in_=psw[w][0:D, :]
                        )
                    prefetch_s(g)
                    pipe_step(g)
                # flush the tail pipeline
                max_off = max(off for off, _ in stages)
                for g in range(len(GROUPS), len(COL_TILES) + max_off):
                    pipe_step(g)

            # ---------------- conv chain
            # conv1 prologue: mlp_m(xi0) + AGs for relation a, pipelined
            for g in range(len(COL_TILES) + 1):
                if g < len(COL_TILES):
                    mlp_m1_ct(g)
                if g >= 1:
                    mlp_m2_ct(g - 1)
                if g in (6, 12, 18):
                    fire_ag("a", g // 6 - 1)
                elif g == 25:
                    fire_ag("a", 3)

            # stages: oldest (surely-ready) PE work first, freshest last
            def mk_stages(with_mlp_m):
                if with_mlp_m:
                    return [(3, mlp_m2_ct), (2, mlp_m1_ct), (1, mlp_u2_ct),
                            (0, mlp_u1_ct)]
                return [(2, h2o_ct), (1, mlp_u2_ct), (0, mlp_u1_ct)]

            # next conv's AG_q fires once mlp_m2 of ct 6q+5 (q<3) / 24 emitted
            def mk_ag(rel):
                return {8: lambda: fire_ag(rel, 0), 14: lambda: fire_ag(rel, 1),
                        20: lambda: fire_ag(rel, 2), 27: lambda: fire_ag(rel, 3)}

            conv(meta_a, "a", mk_stages(True), mk_ag("b"))
            conv(meta_b, "b", mk_stages(True), mk_ag("a"))
            conv(meta_a, "a", mk_stages(False), {})

    nc.compile()
    return nc


# ---------------------------------------------------------------- entry

def _prepare(
    x_served,
    x_interfered,
    edge_s2i,
    edge_i2s,
    wm1,
    bm1,
    wm2,
    bm2,
    wu1,
    bu1,
    wu2,
    bu2,
    wo,
    bo,
):
    """Host prep + program build. Returns (nc, in_maps)."""
    import ml_dtypes

    x_interfered = np.asarray(x_interfered, np.float32)
    e_s2i = np.asarray(edge_s2i)
    e_i2s = np.asarray(edge_i2s)

    # relation a: i2s (src interfered, dst served) -- convs 1 and 3
    meta_a = _prep_relation(e_i2s[0], e_i2s[1])
    # relation b: s2i (src served, dst interfered) -- conv 2
    meta_b = _prep_relation(e_s2i[0], e_s2i[1])

    nc = _build_program(meta_a, meta_b)

    bf = ml_dtypes.bfloat16
    wob = np.concatenate([wo, bo[None, :]], axis=0).astype(bf)

    in_maps = []
    for p in range(NCORES):
        xi_loc = np.zeros((D + 1, PADPER), bf)
        xi_loc[:D, :PERCORE] = x_interfered[p * PERCORE : (p + 1) * PERCORE].T.astype(
            bf
        )
        xi_loc[D, :] = bf(1.0)
        m = {
            "xi0T": xi_loc,
            "wm1": np.ascontiguousarray(np.asarray(wm1).astype(bf)),
            "bm1": np.ascontiguousarray(np.asarray(bm1, np.float32).reshape(HM, 1)),
            "wm2": np.ascontiguousarray(np.asarray(wm2).astype(bf)),
            "bm2r": np.ascontiguousarray(np.asarray(bm2).astype(bf).reshape(1, D)),
            "wu1": np.ascontiguousarray(np.asarray(wu1).astype(bf)),
            "bu1": np.ascontiguousarray(np.asarray(bu1, np.float32).reshape(HU, 1)),
            "wu2": np.ascontiguousarray(np.asarray(wu2).astype(bf)),
            "bu2": np.ascontiguousarray(np.asarray(bu2, np.float32).reshape(D, 1)),
            "wob": wob,
        }
        for rel, meta in (("a", meta_a), ("b", meta_b)):
            for qq in range(NQ):
                m[f"idx_{rel}{qq}"] = meta["idx"][p][qq]
                m[f"drel_{rel}{qq}"] = meta["drel"][p][qq]
        in_maps.append(m)

    return nc, in_maps


def kernel(**inputs):
    from concourse.bass_utils import run_bass_kernel_spmd

    nc, in_maps = _prepare(**inputs)
    res = run_bass_kernel_spmd(
        nc, in_maps, core_ids=list(range(NCORES)), trace=TRACE
    )
    global LAST_RESULT
    LAST_RESULT = res
    outs = [res.results[p]["out"][:PERCORE] for p in range(NCORES)]
    return np.concatenate(outs, axis=0)


# revision 33
# speedup vs baseline: 1.2214x; 1.2214x over previous
"""FDGNN (gnn_message_passing) Trainium2 kernel, 8-core SPMD.

Strategy (v2 — deep pipelining):
- Only 3 of the reference's 6 convs feed the output:
    s1 = conv_i2s(xi0); i2 = conv_s2i(s1); s3 = conv_i2s(i2); out = tanh(s3@wo+bo)
- mlp_m commutes with the per-edge gather: mlp_m(x[src]) == mlp_m(x)[src], so
  the message MLP runs per *node* (12.5K rows/core), not per *edge*.
- Destination nodes are sharded across the 8 cores.
- The message table is split into 4 source-quarters (window-aligned, int16-
  indexable); each quarter is AllGathered separately so gathers for quarter q
  start while later quarters are still in flight.
- Segment-sum: a window-group of 24-26 dst windows stays resident in PSUM
  while all 4 quarter streams accumulate into it:
      psum_w[feat(dup), dst128] += gathered_tile.T @ S_tile
  S_tile is a 0/1 dst-selection matrix built on DVE via iota compare.
- After each window-group: mlp_u for those columns, the NEXT conv's mlp_m for
  the same columns, and the next conv's quarter-AllGather — so collectives and
  MLPs hide behind the next group's gather/matmul work.
- MLPs run in bf16 (fp32 PSUM accumulate).
"""

import numpy as np
import os as _os

NCORES = 8
NNODE = 100000  # both NS and NI
PERCORE = NNODE // NCORES  # 12500
NW = 98  # windows per core (98*128 = 12544)
PADPER = NW * 128  # 12544 padded rows per core
NQ = 4  # source quarters (table chunks)
QSTART = [0, 3072, 6144, 9216, 12544]
QSIZE = [3072, 3072, 3072, 3328]
CHUNK8 = [8 * s for s in QSIZE]  # per-quarter table rows (< 32768, int16-safe)
# dst window-groups kept resident in PSUM (PSUM is bank-granular: 1 bank per
# [128,128] fp32 tile, 8 banks total -> 4 windows/group = one 512-col tile)
GROUPS = [(4 * g, 4 * g + 4) for g in range(24)] + [(96, 98)]
CT_OF_GROUP = [(g, g + 1) for g in range(25)]
AG_AFTER_GROUP = {5: 0, 11: 1, 17: 2, 24: 3}  # group -> quarter to AllGather
CT_RANGE = [(0, 6), (6, 12), (12, 18), (18, 25)]  # 512-col tiles per quarter
D = 64
HM = 32
HU = 16

GT = int(_os.environ.get("KGT", "16"))  # tiles per dma_gather call
SB = 8  # tiles per S-build batch

TRACE = False  # set by test harness to capture an NTFF profile
LAST_RESULT = None  # BassKernelResults of the most recent run

# PADPER = 12544 = 24*512 + 256
COL_TILES = [(i * 512, 512) for i in range(PADPER // 512)]
if PADPER % 512:
    COL_TILES.append((PADPER - PADPER % 512, PADPER % 512))


# ---------------------------------------------------------------- host prep

def _prep_relation(src, dst):
    """Route edges (dst-sharded) into per-core, per-quarter gather streams."""
    E = src.shape[0]
    src = src.astype(np.int64)
    dst = dst.astype(np.int64)
    qstart = np.asarray(QSTART[:NQ])
    qsize = np.asarray(QSIZE)

    p = dst // PERCORE
    dl = dst - p * PERCORE
    w = dl >> 7
    drel = dl & 127

    sl = src % PERCORE
    sc = src // PERCORE
    q = np.searchsorted(QSTART[1:], sl, side="right")
    lidx = sc * qsize[q] + (sl - qstart[q])

    key = (p * NQ + q) * NW + w
    counts = np.bincount(key, minlength=NCORES * NQ * NW).reshape(NCORES, NQ, NW)
    ntiles_qw = -(-counts.max(axis=0) // 128)  # [NQ, NW]
    ntiles_qw[0] = np.maximum(ntiles_qw[0], 1)  # every window has >=1 tile
    N_qw = ntiles_qw * 128
    base_w = np.zeros((NQ, NW + 1), np.int64)
    base_w[:, 1:] = np.cumsum(N_qw, axis=1)
    T_q = (base_w[:, -1] // 128).astype(np.int64)  # tiles per quarter stream

    # rank of each edge within its (p, q, w) cell
    order = np.argsort(key, kind="stable")
    kk = key[order]
    grp_first = np.r_[True, kk[1:] != kk[:-1]]
    first_pos = np.flatnonzero(grp_first)
    starts = np.repeat(first_pos, np.diff(np.r_[first_pos, E]))
    rank = np.arange(E) - starts
    inv = np.empty(E, np.int64)
    inv[order] = rank
    slot = base_w[q, w] + inv  # slot within (core, quarter) stream

    idx_streams = []  # [core][q] -> int16 [128, T_q*8] packed
    drel_streams = []  # [core][q] -> fp32 [128, T_q]
    for pp in range(NCORES):
        rows_i = []
        rows_d = []
        pm = p == pp
        for qq in range(NQ):
            n = int(T_q[qq]) * 128
            ar = np.arange(n)
            idx_flat = (ar * 197) % CHUNK8[qq]  # finite dummy rows, spread
            drel_flat = np.full(n, -1.0, np.float32)
            m = pm & (q == qq)
            idx_flat[slot[m]] = lidx[m]
            drel_flat[slot[m]] = drel[m]
            assert idx_flat.max() < CHUNK8[qq] and idx_flat.min() >= 0
            idx16 = idx_flat.astype(np.int16)
            packed = np.tile(idx16.reshape(n // 16, 16).T, (8, 1))  # [128, n/16]
            rows_i.append(np.ascontiguousarray(packed))
            rows_d.append(
                np.ascontiguousarray(drel_flat.reshape(-1, 128).T.astype(np.float32))
            )
        idx_streams.append(rows_i)
        drel_streams.append(rows_d)

    return {
        "ntiles_qw": ntiles_qw,  # [NQ, NW]
        "T_q": T_q,  # [NQ]
        "idx": idx_streams,
        "drel": drel_streams,
    }


# ---------------------------------------------------------------- program

def _build_program(meta_a, meta_b):
    """meta_a: i2s relation (convs 1 and 3), meta_b: s2i relation (conv 2)."""
    import concourse.mybir as mybir
    import concourse.tile as tile
    from concourse import bacc
    from concourse.bass import ts
    import ml_dtypes

    FP32 = mybir.dt.float32
    BF16 = mybir.dt.bfloat16
    I16 = mybir.dt.int16
    AF = mybir.ActivationFunctionType

    nc = bacc.Bacc(
        "TRN2",
        target_bir_lowering=False,
        debug=False,
        enable_asserts=False,
        num_devices=NCORES,
        num_swdge_queues=4,
        dynamic_dma_scratch_size=49152,
    )

    # ---- I/O (weights bf16, biases fp32)
    xi0T = nc.dram_tensor("xi0T", [D + 1, PADPER], BF16, kind="ExternalInput")
    wm1 = nc.dram_tensor("wm1", [D, HM], BF16, kind="ExternalInput")
    bm1 = nc.dram_tensor("bm1", [HM, 1], FP32, kind="ExternalInput")
    wm2 = nc.dram_tensor("wm2", [HM, D], BF16, kind="ExternalInput")
    bm2r = nc.dram_tensor("bm2r", [1, D], BF16, kind="ExternalInput")
    wu1 = nc.dram_tensor("wu1", [D, HU], BF16, kind="ExternalInput")
    bu1 = nc.dram_tensor("bu1", [HU, 1], FP32, kind="ExternalInput")
    wu2 = nc.dram_tensor("wu2", [HU, D], BF16, kind="ExternalInput")
    bu2 = nc.dram_tensor("bu2", [D, 1], FP32, kind="ExternalInput")
    wob = nc.dram_tensor("wob", [D + 1, D], BF16, kind="ExternalInput")

    idx_in = {}
    drel_in = {}
    for rel, meta in (("a", meta_a), ("b", meta_b)):
        for qq in range(NQ):
            tqn = int(meta["T_q"][qq])
            idx_in[rel, qq] = nc.dram_tensor(
                f"idx_{rel}{qq}", [128, tqn * 8], I16, kind="ExternalInput"
            )
            drel_in[rel, qq] = nc.dram_tensor(
                f"drel_{rel}{qq}", [128, tqn], FP32, kind="ExternalInput"
            )

    out = nc.dram_tensor("out", [PADPER, D], FP32, kind="ExternalOutput")

    # collective buffers; rows hold the 64 bf16 features twice (256B granule)
    y_bounce = nc.dram_tensor("y_bounce", [PADPER, 2 * D], BF16)
    tables = {
        rel: [
            nc.dram_tensor(f"table_{rel}{q}", [CHUNK8[q], 2 * D], BF16,
                           addr_space="Shared")
            for q in range(NQ)
        ]
        for rel in ("a", "b")
    }

    iota_np = np.tile(np.arange(128, dtype=np.float32), (128, 1))
    iota_dram = nc.inline_tensor(iota_np, name="iota")
    onescol_dram = nc.inline_tensor(
        np.ones((1, 512), ml_dtypes.bfloat16), name="onescol"
    )
    zeros_dram = nc.inline_tensor(
        np.zeros((44, 2 * D), ml_dtypes.bfloat16), name="zerorows"
    )

    with tile.TileContext(nc) as tc:
        with (
            tc.tile_pool(name="consts", bufs=1) as cs,
            tc.tile_pool(name="state", bufs=1) as st,
            tc.tile_pool(name="stage", bufs=3) as sg,
            tc.tile_pool(name="meta", bufs=5) as mp,
            tc.tile_pool(name="g0", bufs=3) as gp0,
            tc.tile_pool(name="g1", bufs=3) as gp1,
            tc.tile_pool(name="g2", bufs=3) as gp2,
            tc.tile_pool(name="g3", bufs=3) as gp3,
            tc.tile_pool(name="spool", bufs=4) as sp,
            tc.tile_pool(name="pw", bufs=1, space="PSUM") as pw,
            tc.tile_pool(name="pa", bufs=1, space="PSUM") as pa,
            tc.tile_pool(name="pb", bufs=2, space="PSUM") as pb,
        ):
            gpools = [gp0, gp1, gp2, gp3]
            # ---- constants
            iota_s = cs.tile([128, 128], FP32)
            nc.sync.dma_start(out=iota_s[:], in_=iota_dram[:, :])
            wm1_s = cs.tile([D, HM], BF16)
            nc.sync.dma_start(out=wm1_s[:], in_=wm1[:, :])
            bm1_s = cs.tile([HM, 1], FP32)
            nc.sync.dma_start(out=bm1_s[:], in_=bm1[:, :])
            wm2_s = cs.tile([HM, D], BF16)
            nc.sync.dma_start(out=wm2_s[:], in_=wm2[:, :])
            bm2r_s = cs.tile([1, D], BF16)
            nc.sync.dma_start(out=bm2r_s[:], in_=bm2r[:, :])
            onescol_s = cs.tile([1, 512], BF16)
            nc.sync.dma_start(out=onescol_s[:], in_=onescol_dram[:, :])
            wu1_s = cs.tile([D, HU], BF16)
            nc.sync.dma_start(out=wu1_s[:], in_=wu1[:, :])
            bu1_s = cs.tile([HU, 1], FP32)
            nc.sync.dma_start(out=bu1_s[:], in_=bu1[:, :])
            wu2_s = cs.tile([HU, D], BF16)
            nc.sync.dma_start(out=wu2_s[:], in_=wu2[:, :])
            bu2_s = cs.tile([D, 1], FP32)
            nc.sync.dma_start(out=bu2_s[:], in_=bu2[:, :])
            wob_s = cs.tile([D + 1, D], BF16)
            nc.sync.dma_start(out=wob_s[:], in_=wob[:, :])

            # ---- persistent state
            xT = st.tile([D + 1, PADPER], BF16)  # row D = ones
            nc.sync.dma_start(out=xT[:], in_=xi0T[:, :])
            aggT = st.tile([D, PADPER], BF16)

            # zero rows of the y bounce buffer (pad rows 12500..12543), once
            zrow = cs.tile([44, 2 * D], BF16)
            nc.sync.dma_start(out=zrow[:], in_=zeros_dram[:, :])
            nc.sync.dma_start(out=y_bounce[12500:PADPER, :], in_=zrow[:])

            h1_of = {}

            def mlp_m1_ct(ct):
                """First half of mlp_m for col tile ct: h1 = relu(wm1.T@x+b)."""
                c0, cn = COL_TILES[ct]
                ps = pa.tile([HM, 512], FP32, tag="pa")
                nc.tensor.matmul(
                    ps[:, :cn], wm1_s[:], xT[0:D, c0 : c0 + cn],
                    start=True, stop=True,
                )
                h1 = sg.tile([HM, 512], BF16, tag="h1")
                nc.scalar.activation(h1[:, :cn], ps[:, :cn], AF.Relu, bias=bm1_s[:])
                h1_of[ct] = h1

            def mlp_m2_ct(ct):
                """Second half: y_bounce windows of ct (row-major via flip)."""
                c0, cn = COL_TILES[ct]
                h1 = h1_of.pop(ct)
                for j0 in range(0, cn, 128):
                    j = (c0 + j0) // 128
                    ps2 = pb.tile([128, D], FP32, tag="pb")
                    # bias via rank-1 outer product: ones[128] x bm2[64]
                    nc.tensor.matmul(
                        ps2[:], onescol_s[:, 0:128], bm2r_s[:],
                        start=True, stop=False,
                    )
                    nc.tensor.matmul(
                        ps2[:], h1[:, j0 : j0 + 128], wm2_s[:],
                        start=False, stop=True,
                    )
                    ydup = sg.tile([128, 2 * D], BF16, tag="ydup")
                    nc.scalar.activation(ydup[:, 0:D], ps2[:], AF.Relu)
                    nc.scalar.activation(ydup[:, D : 2 * D], ps2[:], AF.Relu)
                    r0 = j * 128
                    nrows = 128 if j < NW - 1 else (12500 - r0)
                    nc.sync.dma_start(
                        out=y_bounce[r0 : r0 + nrows, :], in_=ydup[0:nrows, :]
                    )

            def fire_ag(rel, q):
                nc.gpsimd.collective_compute(
                    "AllGather",
                    mybir.AluOpType.bypass,
                    replica_groups=[list(range(NCORES))],
                    ins=[y_bounce[QSTART[q] : QSTART[q + 1], :].opt()],
                    outs=[tables[rel][q].ap().opt()],
                )

            hu_of = {}

            def mlp_u1_ct(ct):
                """hu = relu(wu1.T @ aggT[ct cols] + bu1)."""
                c0, cn = COL_TILES[ct]
                ps1 = pa.tile([HU, 512], FP32, tag="pa")
                nc.tensor.matmul(
                    ps1[:, :cn], wu1_s[:], aggT[:, c0 : c0 + cn],
                    start=True, stop=True,
                )
                hu = sg.tile([HU, 512], BF16, tag="hu")
                nc.scalar.activation(hu[:, :cn], ps1[:, :cn], AF.Relu, bias=bu1_s[:])
                hu_of[ct] = hu

            def mlp_u2_ct(ct):
                """xT[:, ct cols] = relu(wu2.T @ hu + bu2)."""
                c0, cn = COL_TILES[ct]
                hu = hu_of.pop(ct)
                ps2 = pa.tile([D, 512], FP32, tag="pu2")
                nc.tensor.matmul(
                    ps2[:, :cn], wu2_s[:], hu[:, :cn], start=True, stop=True
                )
                nc.scalar.activation(
                    xT[0:D, c0 : c0 + cn], ps2[:, :cn], AF.Relu, bias=bu2_s[:]
                )

            def h2o_ct(ct):
                c0, cn = COL_TILES[ct]
                for j0 in range(0, cn, 128):
                    j = (c0 + j0) // 128
                    ps = pb.tile([128, D], FP32, tag="pb")
                    nc.tensor.matmul(
                        ps[:], xT[:, ts(j, 128)], wob_s[:], start=True, stop=True
                    )
                    ostage = sg.tile([128, D], FP32, tag="ostage")
                    nc.scalar.activation(ostage[:], ps[:], AF.Tanh)
                    nc.sync.dma_start(out=out[ts(j, 128), :], in_=ostage[:])

            call_counter = [0]

            def conv(meta, rel, stages, ag_at):
                """Gather + segment-sum -> aggT, with software-pipelined tails.

                stages: list of (offset, fn); after group g emit fn(g - offset)
                        in the given order (put oldest/ready work first).
                ag_at: dict {pipe_step: thunk} firing next conv's AllGathers.
                """

                def pipe_step(g):
                    for off, fn in stages:
                        ct = g - off
                        if 0 <= ct < len(COL_TILES):
                            fn(ct)
                    if g in ag_at:
                        ag_at[g]()
                ntiles_qw = meta["ntiles_qw"]
                T_q = meta["T_q"]
                tbl = tables[rel]

                drel_s = []
                for qq in range(NQ):
                    tqn = int(T_q[qq])
                    dt_ = mp.tile([128, tqn], FP32, tag=f"drel{qq}")
                    nc.sync.dma_start(out=dt_[:], in_=drel_in[rel, qq][:, :])
                    drel_s.append(dt_)

                calls = []
                for qq in range(NQ):
                    tqn = int(T_q[qq])
                    calls.append(
                        [(t0, min(GT, tqn - t0)) for t0 in range(0, tqn, GT)]
                    )

                # per-quarter cumulative tile needs through each group
                need = np.zeros((NQ, len(GROUPS)), np.int64)
                for g, (w0, w1) in enumerate(GROUPS):
                    for qq in range(NQ):
                        need[qq, g] = ntiles_qw[qq, w0:w1].sum()
                need = np.cumsum(need, axis=1)

                ghandles = [dict() for _ in range(NQ)]
                shandles = [dict() for _ in range(NQ)]
                ixhandles = [dict() for _ in range(NQ)]
                next_call = [0] * NQ
                next_ix = [0] * NQ
                next_sb = [0] * NQ

                def issue_idx(qq):
                    k = next_ix[qq]
                    t0, nt = calls[qq][k]
                    ix = mp.tile([128, nt * 8], I16, tag=f"idx{qq}")
                    nc.sync.dma_start(
                        out=ix[:],
                        in_=idx_in[rel, qq][:, t0 * 8 : (t0 + nt) * 8],
                    )
                    ixhandles[qq][k] = ix
                    ixhandles[qq].pop(k - 5, None)
                    next_ix[qq] = k + 1

                def issue_gather(qq):
                    k = next_call[qq]
                    t0, nt = calls[qq][k]
                    ix = ixhandles[qq][k]
                    gb = gpools[qq].tile([128, nt, 2 * D], BF16, tag=f"gb{qq}")
                    nc.gpsimd.dma_gather(
                        gb[:],
                        tbl[qq][:, :],
                        ix[:],
                        nt * 128,
                        nt * 128,
                        2 * D,
                        elem_step=2 * D,
                        queue_num=call_counter[0] % 4,
                        single_packet=False,
                    )
                    call_counter[0] += 1
                    ghandles[qq][k] = gb
                    ghandles[qq].pop(k - 4, None)
                    next_call[qq] = k + 1

                def issue_s(qq):
                    k = next_sb[qq]
                    t0 = k * SB
                    nb = min(SB, int(T_q[qq]) - t0)
                    stile = sp.tile([128, SB, 128], BF16, tag=f"sb{qq}")
                    nc.vector.tensor_tensor(
                        out=stile[:, 0:nb, :],
                        in0=drel_s[qq][:, t0 : t0 + nb].to_broadcast(
                            [128, nb, 128]
                        ),
                        in1=iota_s[:]
                        .rearrange("p (o w) -> p o w", o=1)
                        .to_broadcast([128, nb, 128]),
                        op=mybir.AluOpType.is_equal,
                    )
                    shandles[qq][k] = stile
                    shandles[qq].pop(k - 5, None)
                    next_sb[qq] = k + 1

                def prefetch_gathers(g):
                    tgt_g = min(g + 1, len(GROUPS) - 1)
                    tgt_ix = min(g + 2, len(GROUPS) - 1)
                    for qq in range(NQ):
                        while (
                            next_ix[qq] < len(calls[qq])
                            and next_ix[qq] * GT < need[qq, tgt_ix] + GT
                        ):
                            issue_idx(qq)
                        while (
                            next_call[qq] < len(calls[qq])
                            and next_call[qq] * GT < need[qq, tgt_g]
                        ):
                            issue_gather(qq)

                def prefetch_s(g):
                    tgt_g = min(g + 1, len(GROUPS) - 1)
                    nbat = [
                        -(-int(T_q[qq]) // SB) for qq in range(NQ)
                    ]
                    for qq in range(NQ):
                        while (
                            next_sb[qq] < nbat[qq]
                            and next_sb[qq] * SB < need[qq, tgt_g]
                        ):
                            issue_s(qq)

                prefetch_s(-1)  # group-0 S batches
                tile_cursor = [0] * NQ
                for g, (w0, w1) in enumerate(GROUPS):
                    prefetch_gathers(g)
                    psw = {
                        w: pw.tile(
                            [128, 128], FP32, tag=f"ps{w - w0}",
                            name=f"psw{w - w0}",
                        )
                        for w in range(w0, w1)
                    }
                    total = {
                        w: int(ntiles_qw[:, w].sum()) for w in range(w0, w1)
                    }
                    mm = {w: 0 for w in range(w0, w1)}
                    for qq in range(NQ):
                        t = tile_cursor[qq]
                        for w in range(w0, w1):
                            for _ in range(int(ntiles_qw[qq, w])):
                                gk = t // GT
                                gb = ghandles[qq][gk]
                                gslot = t - calls[qq][gk][0]
                                sk = t // SB
                                stile = shandles[qq][sk]
                                sslot = t - sk * SB
                                nc.tensor.matmul(
                                    psw[w][:],
                                    gb[:, gslot, :],
                                    stile[:, sslot, :],
                                    start=(mm[w] == 0),
                                    stop=(mm[w] == total[w] - 1),
                                )
                                t += 1
                                mm[w] += 1
                        tile_cursor[qq] = t
                    for w in range(w0, w1):
                        nc.vector.tensor_copy(
                            out=aggT[:, ts(w, 128)], in_=psw[w][0:D, :]
                        )
                    prefetch_s(g)
                    pipe_step(g)
                # flush the tail pipeline
                max_off = max(off for off, _ in stages)
                for g in range(len(GROUPS), len(COL_TILES) + max_off):
                    pipe_step(g)

            # ---------------- conv chain
            # conv1 prologue: mlp_m(xi0) + AGs for relation a, pipelined
            for g in range(len(COL_TILES) + 1):
                if g < len(COL_TILES):
                    mlp_m1_ct(g)
                if g >= 1:
                    mlp_m2_ct(g - 1)
                if g in (6, 12, 18):
                    fire_ag("a", g // 6 - 1)
                elif g == 25:
                    fire_ag("a", 3)

            # stages: oldest (surely-ready) PE work first, freshest last
            def mk_stages(with_mlp_m):
                if with_mlp_m:
                    return [(3, mlp_m2_ct), (2, mlp_m1_ct), (1, mlp_u2_ct),
                            (0, mlp_u1_ct)]
                return [(2, h2o_ct), (1, mlp_u2_ct), (0, mlp_u1_ct)]

            # next conv's AG_q fires once mlp_m2 of ct 6q+5 (q<3) / 24 emitted
            def mk_ag(rel):
                return {8: lambda: fire_ag(rel, 0), 14: lambda: fire_ag(rel, 1),
                        20: lambda: fire_ag(rel, 2), 27: lambda: fire_ag(rel, 3)}

            conv(meta_a, "a", mk_stages(True), mk_ag("b"))
            conv(meta_b, "b", mk_stages(True), mk_ag("a"))
            conv(meta_a, "a", mk_stages(False), {})

    nc.compile()
    return nc


# ---------------------------------------------------------------- entry

def _prepare(
    x_served,
    x_interfered,
    edge_s2i,
    edge_i2s,
    wm1,
    bm1,
    wm2,
    bm2,
    wu1,
    bu1,
    wu2,
    bu2,
    wo,
    bo,
):
    """Host prep + program build. Returns (nc, in_maps)."""
    import ml_dtypes

    x_interfered = np.asarray(x_interfered, np.float32)
    e_s2i = np.asarray(edge_s2i)
    e_i2s = np.asarray(edge_i2s)

    # relation a: i2s (src interfered, dst served) -- convs 1 and 3
    meta_a = _prep_relation(e_i2s[0], e_i2s[1])
    # relation b: s2i (src served, dst interfered) -- conv 2
    meta_b = _prep_relation(e_s2i[0], e_s2i[1])

    nc = _build_program(meta_a, meta_b)

    bf = ml_dtypes.bfloat16
    wob = np.concatenate([wo, bo[None, :]], axis=0).astype(bf)

    in_maps = []
    for p in range(NCORES):
        xi_loc = np.zeros((D + 1, PADPER), bf)
        xi_loc[:D, :PERCORE] = x_interfered[p * PERCORE : (p + 1) * PERCORE].T.astype(
            bf
        )
        xi_loc[D, :] = bf(1.0)
        m = {
            "xi0T": xi_loc,
            "wm1": np.ascontiguousarray(np.asarray(wm1).astype(bf)),
            "bm1": np.ascontiguousarray(np.asarray(bm1, np.float32).reshape(HM, 1)),
            "wm2": np.ascontiguousarray(np.asarray(wm2).astype(bf)),
            "bm2r": np.ascontiguousarray(np.asarray(bm2).astype(bf).reshape(1, D)),
            "wu1": np.ascontiguousarray(np.asarray(wu1).astype(bf)),
            "bu1": np.ascontiguousarray(np.asarray(bu1, np.float32).reshape(HU, 1)),
            "wu2": np.ascontiguousarray(np.asarray(wu2).astype(bf)),
            "bu2": np.ascontiguousarray(np.asarray(bu2, np.float32).reshape(D, 1)),
            "wob": wob,
        }
        for rel, meta in (("a", meta_a), ("b", meta_b)):
            for qq in range(NQ):
                m[f"idx_{rel}{qq}"] = meta["idx"][p][qq]
                m[f"drel_{rel}{qq}"] = meta["drel"][p][qq]
        in_maps.append(m)

    return nc, in_maps


def kernel(**inputs):
    from concourse.bass_utils import run_bass_kernel_spmd

    nc, in_maps = _prepare(**inputs)
    res = run_bass_kernel_spmd(
        nc, in_maps, core_ids=list(range(NCORES)), trace=TRACE
    )
    global LAST_RESULT
    LAST_RESULT = res
    outs = [res.results[p]["out"][:PERCORE] for p in range(NCORES)]
    return np.concatenate(outs, axis=0)
